# revision 35
# baseline (speedup 1.0000x reference)
"""Trainium2 Bass kernel for nn_Attention_62130996904205 (v2).

Reference:
    q = left @ Wq;  k,v = split(right @ Wkv)
    per head: S = scale * q k^T; S = where(mask, S, -1e7)
    out = (softmax(S) @ v) rearranged @ Wout + bout

Sharding: 8 cores = (batch 0..3) x (head-half 0..1); each core handles 4
heads of one batch, host sums the two head-half partial out-projections
and adds bout.

v2 on-chip scheme ("S^T layout + PE tiling + multi-engine softmax"):
  - S^T tiles: psum[128 kv-tok, 512 m] per head; two heads computed
    CONCURRENTLY via PE row tiling (K=64 each, tile_position (0,0) and
    (64,0)) into the two banks of one [128,1024] psum tile.
  - exp: scale folded so psum holds y/16; three engine paths per nt,
    statically scheduled: scalar ACT exp(scale=16); custom DVE op
    EXPQ16_ANT ((s(x+p)^2+t)^16); mask multiply on DVE or GpSimd.
  - O^T = v^T @ P^T per head; two heads CONCURRENT via PE col tiling
    (64-wide v, tile_position (0,0)/(0,64)) accumulating into one
    [128, 512] psum bank across all 32 kv tiles.
  - softmax denominators via ones-vector matmuls, 4-way col-tiled
    (positions 0/32/64/96) into one psum bank.
  - normalize with reciprocal_approx_fast + gpsimd partition_broadcast;
    out-projection from normalized U tiles, interleaved/tail.
"""

import numpy as np
import ml_dtypes

import concourse.bass as bass
import concourse.mybir as mybir
import concourse.tile as tile
from concourse import bacc
from concourse.bass_utils import run_bass_kernel_spmd

BF16 = ml_dtypes.bfloat16
FP32 = np.float32

TRACE = False
DEBUG = False
LAST_RESULTS = None

def build_core(M=1024, N=4096, DQ=512, H=4, DH=64):
    dt = mybir.dt
    f32, bf16 = dt.float32, dt.bfloat16
    D = H * DH            # 256 features on this core
    KT = DQ // 128        # 4 contraction tiles for projections
    NT = N // 128         # 32 kv-token tiles
    MCH = 512             # m chunk
    NMC = M // MCH        # 2
    NP = H // 2           # head pairs
    NKC = 4               # k-projection col chunks of 1024
    EXP = mybir.ActivationFunctionType.Exp

    nc = bacc.Bacc("TRN2", target_bir_lowering=False, debug=False)

    leftT = nc.dram_tensor("leftT", [DQ, M], bf16, kind="ExternalInput")
    rightT = nc.dram_tensor("rightT", [DQ, N], bf16, kind="ExternalInput")
    maskT = nc.dram_tensor("maskT", [N, M], bf16, kind="ExternalInput")
    wq = nc.dram_tensor("wq", [DQ, D], bf16, kind="ExternalInput")
    wk = nc.dram_tensor("wk", [DQ, D], bf16, kind="ExternalInput")
    wv = nc.dram_tensor("wv", [DQ, D], bf16, kind="ExternalInput")
    wout = nc.dram_tensor("wout", [D, DQ], bf16, kind="ExternalInput")
    out_p = nc.dram_tensor("out_p", [M, DQ], f32, kind="ExternalOutput")
    if DEBUG:
        dbg_d = nc.dram_tensor("dbg_d", [NMC * NP * 2, M // NMC], f32, kind="ExternalOutput")
        dbg_o = nc.dram_tensor("dbg_o", [NMC * NP * 2, M // NMC], f32, kind="ExternalOutput")
        dbg_u = nc.dram_tensor("dbg_u", [128, M], bf16, kind="ExternalOutput")
        dbg_q = nc.dram_tensor("dbg_q", [128, M], bf16, kind="ExternalOutput")
        dbg_k = nc.dram_tensor("dbg_k", [128, N], bf16, kind="ExternalOutput")
        dbg_pm = nc.dram_tensor("dbg_pm", [128, 2 * (M // NMC)], bf16, kind="ExternalOutput")

    with tile.TileContext(nc) as tc:
        with (
            tc.tile_pool(name="sing", bufs=1) as sing,
            tc.tile_pool(name="mskp", bufs=5) as mskp,
            tc.tile_pool(name="spool", bufs=3, space="PSUM") as spool,
            tc.tile_pool(name="opool", bufs=1, space="PSUM") as opool,
            tc.tile_pool(name="dpool", bufs=1, space="PSUM") as dpool,
            tc.tile_pool(name="ppool", bufs=4) as ppool,
            tc.tile_pool(name="pmp", bufs=11) as pmp,
            tc.tile_pool(name="smallp", bufs=2) as smallp,
            tc.tile_pool(name="bpool", bufs=2) as bpool,
            tc.tile_pool(name="outp", bufs=3) as outp,
        ):
            # ---------------- input DMA ---------------------------------
            wq_sb = sing.tile([128, KT, D], bf16, tag="wq")
            nc.gpsimd.dma_start(out=wq_sb, in_=wq.rearrange("(kt p) d -> p kt d", p=128))
            wk_sb = sing.tile([128, KT, D], bf16, tag="wk")
            nc.gpsimd.dma_start(out=wk_sb, in_=wk.rearrange("(kt p) d -> p kt d", p=128))
            wv_sb = sing.tile([128, KT, D], bf16, tag="wv")
            nc.gpsimd.dma_start(out=wv_sb, in_=wv.rearrange("(kt p) d -> p kt d", p=128))
            leftT_sb = []
            for kt in range(KT):
                t = sing.tile([128, M], bf16, tag=f"leftT{kt}", name=f"leftT{kt}")
                nc.gpsimd.dma_start(out=t, in_=leftT[kt * 128 : (kt + 1) * 128, :])
                leftT_sb.append(t)
            rightT_sb = [
                sing.tile([128, N], bf16, tag=f"rightT{kt}", name=f"rightT{kt}")
                for kt in range(KT)
            ]
            # column-major waves so k-proj chunk c has all kt tiles early;
            # wave 1 rides the gpsimd queue so sync is free for mc0 masks.
            for kt in range(KT):
                nc.sync.dma_start(
                    out=rightT_sb[kt][:, 0:2048],
                    in_=rightT[kt * 128 : (kt + 1) * 128, 0:2048],
                )
            for kt in range(KT):
                nc.gpsimd.dma_start(
                    out=rightT_sb[kt][:, 2048:4096],
                    in_=rightT[kt * 128 : (kt + 1) * 128, 2048:4096],
                )
            wout_sb = sing.tile([128, D // 128, DQ], bf16, tag="wout")
            nc.gpsimd.dma_start(
                out=wout_sb, in_=wout.rearrange("(kt p) d -> p kt d", p=128)
            )
            ones = sing.tile([128, 1], bf16, tag="ones")
            nc.vector.memset(ones, 1.0)

            qT2 = [sing.tile([128, M], bf16, tag=f"qT{p}", name=f"qT{p}") for p in range(NP)]
            kT2 = [sing.tile([128, N], bf16, tag=f"kT{p}", name=f"kT{p}") for p in range(NP)]
            v_sb = sing.tile([128, NT, H, DH], bf16, tag="v")
            u_sb = [sing.tile([128, M], bf16, tag=f"u{p}", name=f"u{p}") for p in range(NP)]

            def mask_chunk(mc, c):
                mt = mskp.tile([128, 8, MCH], bf16, tag="msk", name=f"msk{mc}_{c}")
                src = maskT[
                    c * 1024 : (c + 1) * 1024, mc * MCH : (mc + 1) * MCH
                ].rearrange("(a p) f -> p a f", p=128)
                nc.sync.dma_start(out=mt, in_=src)
                return mt

            # ---------------- projections -------------------------------
            def q_proj(p):
                ps = spool.tile([128, 2 * MCH], f32, tag="s", name="qps")
                for mh in range(2):
                    for kt in range(KT):
                        nc.tensor.matmul(
                            ps[:, mh * MCH : (mh + 1) * MCH],
                            lhsT=wq_sb[:, kt, p * 128 : (p + 1) * 128],
                            rhs=leftT_sb[kt][:, mh * MCH : (mh + 1) * MCH],
                            start=(kt == 0),
                            stop=(kt == KT - 1),
                        )
                nc.vector.tensor_copy(out=qT2[p], in_=ps)

            def k_half(p, c, half):
                ps = spool.tile([128, 2 * MCH], f32, tag="s", name="kps")
                for kt in range(KT):
                    nc.tensor.matmul(
                        ps[:, 0:MCH],
                        lhsT=wk_sb[:, kt, p * 128 : (p + 1) * 128],
                        rhs=rightT_sb[kt][
                            :,
                            c * 1024 + half * MCH : c * 1024 + (half + 1) * MCH,
                        ],
                        start=(kt == 0),
                        stop=(kt == KT - 1),
                    )
                lo = c * 1024 + half * MCH
                nc.vector.tensor_copy(
                    out=kT2[p][:, lo : lo + MCH], in_=ps[:, 0:MCH]
                )

            def k_chunk(p, c):
                k_half(p, c, 0)
                k_half(p, c, 1)

            def v_nt(nt):
                ps = spool.tile([128, 2 * MCH], f32, tag="s", name="vps")
                for kt in range(KT):
                    nc.tensor.matmul(
                        ps[:, 0:D],
                        lhsT=rightT_sb[kt][:, nt * 128 : (nt + 1) * 128],
                        rhs=wv_sb[:, kt, :],
                        start=(kt == 0),
                        stop=(kt == KT - 1),
                    )
                nc.vector.tensor_copy(out=v_sb[:, nt, :, :], in_=ps[:, 0:D])

            def outproj_mt(mt):
                ps = spool.tile([128, 2 * MCH], f32, tag="s", name="ops")
                for p2 in range(D // 128):
                    nc.tensor.matmul(
                        ps[:, 0:DQ],
                        lhsT=u_sb[p2][:, mt * 128 : (mt + 1) * 128],
                        rhs=wout_sb[:, p2, :],
                        start=(p2 == 0),
                        stop=(p2 == D // 128 - 1),
                    )
                ob = outp.tile([128, DQ], f32, tag="ob")
                nc.vector.tensor_copy(out=ob, in_=ps[:, 0:DQ])
                nc.gpsimd.dma_start(out=out_p[mt * 128 : (mt + 1) * 128, :], in_=ob)

            # prologue: q, first k chunk, first v tiles
            k_chunk(0, 0)
            q_proj(0)
            q_proj(1)
            for nt in range(16):
                v_nt(nt)

            # deferred projection work: pair-0 k chunks + remaining v tiles
            # paced through (mc0, hp0); pair-1 k chunks through (mc0, hp1).
            deferred = []
            vq = list(range(16, NT))
            for ci, c in enumerate((1, 2, 3)):
                deferred.append(lambda c=c: k_chunk(0, c))
                if ci == 1:
                    deferred.append(lambda: k_chunk(1, 0))
                take, vq = vq[:5], vq[5:]
                for nt in take:
                    deferred.append(lambda nt=nt: v_nt(nt))
            for nt in vq:
                deferred.append(lambda nt=nt: v_nt(nt))
            deferred2 = [lambda c=c: k_chunk(1, c) for c in (1, 2, 3)]
            lazy = []

            # ---------------- attention ---------------------------------
            DEPTH = 8
            prefetched = {}
            for mc in range(NMC):
                msks = [
                    prefetched.pop((mc, c), None) or mask_chunk(mc, c)
                    for c in range(4)
                ]
                for hp in range(NP):
                    o_ps = opool.tile([128, MCH], f32, tag="o")
                    d_ps = dpool.tile([128, MCH], f32, tag="d")
                    oq = []

                    def flush_one(oq=oq, o_ps=o_ps, d_ps=d_ps, hp=hp):
                        pm, nt, pm_prev = oq.pop(0)
                        for i in range(2):
                            nc.tensor.matmul(
                                o_ps[i * 64 : (i + 1) * 64, :],
                                lhsT=v_sb[:, nt, 2 * hp + i, :],
                                rhs=pm[:, i * MCH : (i + 1) * MCH],
                                start=(nt == 0),
                                stop=(nt == NT - 1),
                                tile_position=(0, i * 64),
                            )
                        if nt % 2 == 1:
                            for ci, (src, half) in enumerate(
                                [(pm_prev, 0), (pm_prev, 1), (pm, 0), (pm, 1)]
                            ):
                                c = ci * 32
                                nc.tensor.matmul(
                                    d_ps[c : c + 1, :],
                                    lhsT=ones,
                                    rhs=src[:, half * MCH : (half + 1) * MCH],
                                    start=(nt == 1),
                                    stop=(nt == NT - 1),
                                    tile_position=(0, c),
                                )

                    pm_prev = [None]
                    for nt in range(NT):
                        if mc == 0 and hp == 1 and deferred2 and nt % 2 == 0:
                            deferred2.pop(0)()
                        elif deferred:
                            deferred.pop(0)()
                        elif lazy and nt % 4 == 2:
                            lazy.pop(0)()
                        sp = spool.tile([128, 2 * MCH], f32, tag="s")
                        for i in range(2):
                            nc.tensor.matmul(
                                sp[:, i * MCH : (i + 1) * MCH],
                                lhsT=kT2[hp][i * 64 : (i + 1) * 64, nt * 128 : (nt + 1) * 128],
                                rhs=qT2[hp][i * 64 : (i + 1) * 64, mc * MCH : (mc + 1) * MCH],
                                start=True,
                                stop=True,
                                tile_position=(i * 64, 0),
                            )
                        m_t = msks[nt // 8][:, nt % 8, :]
                        p_t = ppool.tile([128, 2 * MCH], bf16, tag="p")
                        nc.scalar.activation(p_t, sp, EXP)
                        pm = pmp.tile([128, 2 * MCH], bf16, tag="pm")
                        for i in range(2):
                            nc.vector.tensor_mul(
                                pm[:, i * MCH : (i + 1) * MCH],
                                p_t[:, i * MCH : (i + 1) * MCH],
                                m_t,
                            )
                        if DEBUG and mc == 0 and hp == 0 and nt == 0:
                            nc.sync.dma_start(out=dbg_pm[:, :], in_=pm)
                        oq.append((pm, nt, pm_prev[0]))
                        pm_prev[0] = pm
                        depth_now = (
                            2
                            if (mc == NMC - 1 and hp == NP - 1 and nt >= NT - 8)
                            else DEPTH
                        )
                        while len(oq) > depth_now:
                            flush_one()
                    if hp == NP - 1 and mc + 1 < NMC:
                        prefetched[(mc + 1, 0)] = mask_chunk(mc + 1, 0)
                    # drain remaining projection work BEFORE the tail O
                    # flushes: a flush reads v_sb tiles whose projection
                    # must already be emitted (program order = dep order).
                    if mc == 0 and hp == 0:
                        while deferred:
                            deferred.pop(0)()
                    if mc == 0 and hp == 1:
                        while deferred2:
                            deferred2.pop(0)()
                    while oq:
                        flush_one()

                    # normalize: u = O / d  (d rows: 0+64 -> head0, 32+96 -> head1)
                    for i in range(2):
                        c1 = smallp.tile([1, MCH], f32, tag="c1", name=f"c1{i}")
                        nc.vector.tensor_copy(c1, d_ps[64 + i * 32 : 64 + i * 32 + 1, :])
                        ds = smallp.tile([1, MCH], f32, tag="ds", name=f"ds{i}")
                        nc.vector.tensor_add(ds, d_ps[i * 32 : i * 32 + 1, :], c1)
                        rd = smallp.tile([1, MCH], f32, tag="rd", name=f"rd{i}")
                        nc.vector.reciprocal_approx_fast(out=rd, in_=ds)
                        if DEBUG:
                            row = (mc * NP + hp) * 2 + i
                            nc.sync.dma_start(out=dbg_d[row : row + 1, :], in_=ds)
                            oc = smallp.tile([1, MCH], f32, tag="oc", name=f"oc{i}")
                            nc.vector.tensor_copy(oc, o_ps[i * 64 : i * 64 + 1, :])
                            nc.sync.dma_start(out=dbg_o[row : row + 1, :], in_=oc)
                        bd = bpool.tile([64, MCH], f32, tag="bd", name=f"bd{i}")
                        nc.gpsimd.partition_broadcast(bd, rd)
                        nc.vector.tensor_mul(
                            u_sb[hp][i * 64 : (i + 1) * 64, mc * MCH : (mc + 1) * MCH],
                            o_ps[i * 64 : (i + 1) * 64, :],
                            bd,
                        )
                for mt in range(mc * 4, mc * 4 + 4):
                    lazy.append(lambda mt=mt: outproj_mt(mt))
            while deferred:
                deferred.pop(0)()
            while lazy:
                lazy.pop(0)()
            if DEBUG:
                nc.sync.dma_start(out=dbg_u[:, :], in_=u_sb[0])
                nc.sync.dma_start(out=dbg_q[:, :], in_=qT2[0])
                nc.sync.dma_start(out=dbg_k[:, :], in_=kT2[0])

    nc.finalize()
    return nc


_NC_CACHE = {}


def _get_nc(key=(1024, 4096, 512, 4, 64)):
    if key not in _NC_CACHE:
        _NC_CACHE[key] = build_core(*key)
    return _NC_CACHE[key]


def kernel(left, right, mask, Wq, Wkv, Wout, bout):
    global LAST_RESULTS
    B, M, DQmat = left.shape
    _, N, DC = right.shape
    H, DH = 8, 64
    D = H * DH
    Hc = H // 2
    scale = DH ** -0.5

    left = np.asarray(left, dtype=np.float32)
    right = np.asarray(right, dtype=np.float32)
    Wq = np.asarray(Wq, dtype=np.float32)
    Wkv = np.asarray(Wkv, dtype=np.float32)
    Wout = np.asarray(Wout, dtype=np.float32)
    bout = np.asarray(bout, dtype=np.float32)

    Wqs = (Wq * scale).astype(BF16)
    Wk = Wkv[:, :D].astype(BF16)
    Wv = Wkv[:, D:].astype(BF16)
    WoutB = Wout.astype(BF16)

    leftT = np.ascontiguousarray(left.transpose(0, 2, 1)).astype(BF16)
    rightT = np.ascontiguousarray(right.transpose(0, 2, 1)).astype(BF16)
    maskT = np.ascontiguousarray(mask.transpose(0, 2, 1)).astype(BF16)

    nc = _get_nc((M, N, DQmat, Hc, DH))

    in_maps = []
    for core in range(8):
        b, hh = divmod(core, 2)
        hs = slice(hh * Hc * DH, (hh + 1) * Hc * DH)
        in_maps.append(
            {
                "leftT": leftT[b],
                "rightT": rightT[b],
                "maskT": maskT[b],
                "wq": np.ascontiguousarray(Wqs[:, hs]),
                "wk": np.ascontiguousarray(Wk[:, hs]),
                "wv": np.ascontiguousarray(Wv[:, hs]),
                "wout": np.ascontiguousarray(WoutB[hs, :]),
            }
        )

    tmpdir = None
    if TRACE:
        import shutil

        shutil.rmtree("/tmp/attn_trace", ignore_errors=True)
        tmpdir = "/tmp/attn_trace"
    res = run_bass_kernel_spmd(nc, in_maps, list(range(8)), trace=TRACE, tmpdir=tmpdir)
    LAST_RESULTS = res

    out = np.zeros((B, M, DQmat), np.float32)
    for core in range(8):
        out[core // 2] += res.results[core]["out_p"]
    out += bout[None, None, :]
    return out


# revision 36
# speedup vs baseline: 1.0051x; 1.0051x over previous
"""Trainium2 Bass kernel for nn_Attention_62130996904205 (v2).

Reference:
    q = left @ Wq;  k,v = split(right @ Wkv)
    per head: S = scale * q k^T; S = where(mask, S, -1e7)
    out = (softmax(S) @ v) rearranged @ Wout + bout

Sharding: 8 cores = (batch 0..3) x (head-half 0..1); each core handles 4
heads of one batch, host sums the two head-half partial out-projections
and adds bout.

On-chip scheme ("S^T layout + PE tiling"):
  - S^T tiles: psum[128 kv-tok, 512 m] per head; the two heads of a pair
    are computed CONCURRENTLY via PE row tiling (K=64 each, tile_position
    (0,0)/(64,0)) into the two banks of one [128,1024] psum tile.
  - softmax: scalar-engine exp (psum -> bf16 SBUF), then the mask
    multiply as two 512-wide bf16 tensor_muls on the vector engine
    (2x perf mode). All 128 exp tiles ride the scalar engine; it and
    the tensor engine are the matched bottlenecks.
  - O^T = v^T @ P^T per head; two heads CONCURRENT via PE col tiling
    (64-wide v, tile_position (0,0)/(0,64)) accumulating into one
    [128, 512] psum bank across all 32 kv tiles.
  - softmax denominators via ones-vector matmuls, 4-way col-tiled
    (positions 0/32/64/96) into one psum bank, batched over nt pairs.
  - normalize with reciprocal_approx_fast + gpsimd partition_broadcast;
    out-projection from normalized U tiles, interleaved/tail.
  - q/k/v projections are software-pipelined: a prologue covers what the
    first attention phase needs; the rest is paced one unit per nt
    iteration through phases (mc0,hp0) and (mc0,hp1).
  - PSUM: 3x[128,1024] S rotation + O accumulator + denominator = 8 banks.
"""

import numpy as np
import ml_dtypes

import concourse.bass as bass
import concourse.mybir as mybir
import concourse.tile as tile
from concourse import bacc
from concourse.bass_utils import run_bass_kernel_spmd

BF16 = ml_dtypes.bfloat16
FP32 = np.float32

TRACE = False
DEBUG = False
LAST_RESULTS = None

def build_core(M=1024, N=4096, DQ=512, H=4, DH=64):
    dt = mybir.dt
    f32, bf16 = dt.float32, dt.bfloat16
    D = H * DH            # 256 features on this core
    KT = DQ // 128        # 4 contraction tiles for projections
    NT = N // 128         # 32 kv-token tiles
    MCH = 512             # m chunk
    NMC = M // MCH        # 2
    NP = H // 2           # head pairs
    NKC = 4               # k-projection col chunks of 1024
    EXP = mybir.ActivationFunctionType.Exp

    nc = bacc.Bacc("TRN2", target_bir_lowering=False, debug=False)

    leftT = nc.dram_tensor("leftT", [DQ, M], bf16, kind="ExternalInput")
    rightT = nc.dram_tensor("rightT", [DQ, N], bf16, kind="ExternalInput")
    maskT = nc.dram_tensor("maskT", [N, M], bf16, kind="ExternalInput")
    wq = nc.dram_tensor("wq", [DQ, D], bf16, kind="ExternalInput")
    wk = nc.dram_tensor("wk", [DQ, D], bf16, kind="ExternalInput")
    wv = nc.dram_tensor("wv", [DQ, D], bf16, kind="ExternalInput")
    wout = nc.dram_tensor("wout", [D, DQ], bf16, kind="ExternalInput")
    out_p = nc.dram_tensor("out_p", [M, DQ], f32, kind="ExternalOutput")
    if DEBUG:
        dbg_d = nc.dram_tensor("dbg_d", [NMC * NP * 2, M // NMC], f32, kind="ExternalOutput")
        dbg_o = nc.dram_tensor("dbg_o", [NMC * NP * 2, M // NMC], f32, kind="ExternalOutput")
        dbg_u = nc.dram_tensor("dbg_u", [128, M], bf16, kind="ExternalOutput")
        dbg_q = nc.dram_tensor("dbg_q", [128, M], bf16, kind="ExternalOutput")
        dbg_k = nc.dram_tensor("dbg_k", [128, N], bf16, kind="ExternalOutput")
        dbg_pm = nc.dram_tensor("dbg_pm", [128, 2 * (M // NMC)], bf16, kind="ExternalOutput")

    with tile.TileContext(nc) as tc:
        with (
            tc.tile_pool(name="sing", bufs=1) as sing,
            tc.tile_pool(name="mskp", bufs=5) as mskp,
            tc.tile_pool(name="spool", bufs=3, space="PSUM") as spool,
            tc.tile_pool(name="opool", bufs=1, space="PSUM") as opool,
            tc.tile_pool(name="dpool", bufs=1, space="PSUM") as dpool,
            tc.tile_pool(name="ppool", bufs=4) as ppool,
            tc.tile_pool(name="pmp", bufs=11) as pmp,
            tc.tile_pool(name="smallp", bufs=2) as smallp,
            tc.tile_pool(name="bpool", bufs=2) as bpool,
            tc.tile_pool(name="outp", bufs=3) as outp,
        ):
            # ---------------- input DMA ---------------------------------
            wq_sb = sing.tile([128, KT, D], bf16, tag="wq")
            nc.gpsimd.dma_start(out=wq_sb, in_=wq.rearrange("(kt p) d -> p kt d", p=128))
            wk_sb = sing.tile([128, KT, D], bf16, tag="wk")
            nc.gpsimd.dma_start(out=wk_sb, in_=wk.rearrange("(kt p) d -> p kt d", p=128))
            wv_sb = sing.tile([128, KT, D], bf16, tag="wv")
            nc.gpsimd.dma_start(out=wv_sb, in_=wv.rearrange("(kt p) d -> p kt d", p=128))
            leftT_sb = []
            for kt in range(KT):
                t = sing.tile([128, M], bf16, tag=f"leftT{kt}", name=f"leftT{kt}")
                nc.gpsimd.dma_start(out=t, in_=leftT[kt * 128 : (kt + 1) * 128, :])
                leftT_sb.append(t)
            rightT_sb = [
                sing.tile([128, N], bf16, tag=f"rightT{kt}", name=f"rightT{kt}")
                for kt in range(KT)
            ]
            # column-major waves so k-proj chunk c has all kt tiles early;
            # wave 1 rides the gpsimd queue so sync is free for mc0 masks.
            for kt in range(KT):
                nc.sync.dma_start(
                    out=rightT_sb[kt][:, 0:2048],
                    in_=rightT[kt * 128 : (kt + 1) * 128, 0:2048],
                )
            for kt in range(KT):
                nc.gpsimd.dma_start(
                    out=rightT_sb[kt][:, 2048:4096],
                    in_=rightT[kt * 128 : (kt + 1) * 128, 2048:4096],
                )
            wout_sb = sing.tile([128, D // 128, DQ], bf16, tag="wout")
            nc.gpsimd.dma_start(
                out=wout_sb, in_=wout.rearrange("(kt p) d -> p kt d", p=128)
            )
            ones = sing.tile([128, 1], bf16, tag="ones")
            nc.vector.memset(ones, 1.0)

            qT2 = [sing.tile([128, M], bf16, tag=f"qT{p}", name=f"qT{p}") for p in range(NP)]
            kT2 = [sing.tile([128, N], bf16, tag=f"kT{p}", name=f"kT{p}") for p in range(NP)]
            v_sb = sing.tile([128, NT, H, DH], bf16, tag="v")
            u_sb = [sing.tile([128, M], bf16, tag=f"u{p}", name=f"u{p}") for p in range(NP)]

            def mask_chunk(mc, c):
                mt = mskp.tile([128, 8, MCH], bf16, tag="msk", name=f"msk{mc}_{c}")
                src = maskT[
                    c * 1024 : (c + 1) * 1024, mc * MCH : (mc + 1) * MCH
                ].rearrange("(a p) f -> p a f", p=128)
                nc.sync.dma_start(out=mt, in_=src)
                return mt

            # ---------------- projections -------------------------------
            def q_proj(p):
                ps = spool.tile([128, 2 * MCH], f32, tag="s", name="qps")
                for mh in range(2):
                    for kt in range(KT):
                        nc.tensor.matmul(
                            ps[:, mh * MCH : (mh + 1) * MCH],
                            lhsT=wq_sb[:, kt, p * 128 : (p + 1) * 128],
                            rhs=leftT_sb[kt][:, mh * MCH : (mh + 1) * MCH],
                            start=(kt == 0),
                            stop=(kt == KT - 1),
                        )
                nc.vector.tensor_copy(out=qT2[p], in_=ps)

            def k_half(p, c, half):
                ps = spool.tile([128, 2 * MCH], f32, tag="s", name="kps")
                for kt in range(KT):
                    nc.tensor.matmul(
                        ps[:, 0:MCH],
                        lhsT=wk_sb[:, kt, p * 128 : (p + 1) * 128],
                        rhs=rightT_sb[kt][
                            :,
                            c * 1024 + half * MCH : c * 1024 + (half + 1) * MCH,
                        ],
                        start=(kt == 0),
                        stop=(kt == KT - 1),
                    )
                lo = c * 1024 + half * MCH
                nc.vector.tensor_copy(
                    out=kT2[p][:, lo : lo + MCH], in_=ps[:, 0:MCH]
                )

            def k_chunk(p, c):
                k_half(p, c, 0)
                k_half(p, c, 1)

            def v_nt(nt):
                ps = spool.tile([128, 2 * MCH], f32, tag="s", name="vps")
                for kt in range(KT):
                    nc.tensor.matmul(
                        ps[:, 0:D],
                        lhsT=rightT_sb[kt][:, nt * 128 : (nt + 1) * 128],
                        rhs=wv_sb[:, kt, :],
                        start=(kt == 0),
                        stop=(kt == KT - 1),
                    )
                nc.vector.tensor_copy(out=v_sb[:, nt, :, :], in_=ps[:, 0:D])

            def outproj_mt(mt):
                ps = spool.tile([128, 2 * MCH], f32, tag="s", name="ops")
                for p2 in range(D // 128):
                    nc.tensor.matmul(
                        ps[:, 0:DQ],
                        lhsT=u_sb[p2][:, mt * 128 : (mt + 1) * 128],
                        rhs=wout_sb[:, p2, :],
                        start=(p2 == 0),
                        stop=(p2 == D // 128 - 1),
                    )
                ob = outp.tile([128, DQ], f32, tag="ob")
                nc.vector.tensor_copy(out=ob, in_=ps[:, 0:DQ])
                nc.gpsimd.dma_start(out=out_p[mt * 128 : (mt + 1) * 128, :], in_=ob)

            # prologue: q, first k chunk, first v tiles
            k_chunk(0, 0)
            q_proj(0)
            q_proj(1)
            for nt in range(16):
                v_nt(nt)

            # deferred projection work: pair-0 k chunks + remaining v tiles
            # paced through (mc0, hp0); pair-1 k chunks through (mc0, hp1).
            deferred = []
            vq = list(range(16, NT))
            for ci, c in enumerate((1, 2, 3)):
                deferred.append(lambda c=c: k_chunk(0, c))
                if ci == 1:
                    deferred.append(lambda: k_chunk(1, 0))
                take, vq = vq[:5], vq[5:]
                for nt in take:
                    deferred.append(lambda nt=nt: v_nt(nt))
            for nt in vq:
                deferred.append(lambda nt=nt: v_nt(nt))
            deferred2 = [lambda c=c: k_chunk(1, c) for c in (1, 2, 3)]
            lazy = []

            # ---------------- attention ---------------------------------
            DEPTH = 8
            prefetched = {}
            for mc in range(NMC):
                msks = [
                    prefetched.pop((mc, c), None) or mask_chunk(mc, c)
                    for c in range(4)
                ]
                for hp in range(NP):
                    o_ps = opool.tile([128, MCH], f32, tag="o")
                    d_ps = dpool.tile([128, MCH], f32, tag="d")
                    oq = []

                    def flush_one(oq=oq, o_ps=o_ps, d_ps=d_ps, hp=hp):
                        pm, nt, pm_prev = oq.pop(0)
                        for i in range(2):
                            nc.tensor.matmul(
                                o_ps[i * 64 : (i + 1) * 64, :],
                                lhsT=v_sb[:, nt, 2 * hp + i, :],
                                rhs=pm[:, i * MCH : (i + 1) * MCH],
                                start=(nt == 0),
                                stop=(nt == NT - 1),
                                tile_position=(0, i * 64),
                            )
                        if nt % 2 == 1:
                            for ci, (src, half) in enumerate(
                                [(pm_prev, 0), (pm_prev, 1), (pm, 0), (pm, 1)]
                            ):
                                c = ci * 32
                                nc.tensor.matmul(
                                    d_ps[c : c + 1, :],
                                    lhsT=ones,
                                    rhs=src[:, half * MCH : (half + 1) * MCH],
                                    start=(nt == 1),
                                    stop=(nt == NT - 1),
                                    tile_position=(0, c),
                                )

                    pm_prev = [None]
                    for nt in range(NT):
                        if mc == 0 and hp == 1 and deferred2 and nt % 2 == 0:
                            deferred2.pop(0)()
                        elif deferred:
                            deferred.pop(0)()
                        elif lazy and nt % 4 == 2:
                            lazy.pop(0)()
                        sp = spool.tile([128, 2 * MCH], f32, tag="s")
                        for i in range(2):
                            nc.tensor.matmul(
                                sp[:, i * MCH : (i + 1) * MCH],
                                lhsT=kT2[hp][i * 64 : (i + 1) * 64, nt * 128 : (nt + 1) * 128],
                                rhs=qT2[hp][i * 64 : (i + 1) * 64, mc * MCH : (mc + 1) * MCH],
                                start=True,
                                stop=True,
                                tile_position=(i * 64, 0),
                            )
                        m_t = msks[nt // 8][:, nt % 8, :]
                        p_t = ppool.tile([128, 2 * MCH], bf16, tag="p")
                        nc.scalar.activation(p_t, sp, EXP)
                        pm = pmp.tile([128, 2 * MCH], bf16, tag="pm")
                        for i in range(2):
                            nc.vector.tensor_mul(
                                pm[:, i * MCH : (i + 1) * MCH],
                                p_t[:, i * MCH : (i + 1) * MCH],
                                m_t,
                            )
                        if DEBUG and mc == 0 and hp == 0 and nt == 0:
                            nc.sync.dma_start(out=dbg_pm[:, :], in_=pm)
                        oq.append((pm, nt, pm_prev[0]))
                        pm_prev[0] = pm
                        depth_now = (
                            2
                            if (mc == NMC - 1 and hp == NP - 1 and nt >= NT - 8)
                            else DEPTH
                        )
                        while len(oq) > depth_now:
                            flush_one()
                    if hp == NP - 1 and mc + 1 < NMC:
                        prefetched[(mc + 1, 0)] = mask_chunk(mc + 1, 0)
                    # drain remaining projection work BEFORE the tail O
                    # flushes: a flush reads v_sb tiles whose projection
                    # must already be emitted (program order = dep order).
                    if mc == 0 and hp == 0:
                        while deferred:
                            deferred.pop(0)()
                    if mc == 0 and hp == 1:
                        while deferred2:
                            deferred2.pop(0)()
                    while oq:
                        flush_one()

                    # normalize: u = O / d  (d rows: 0+64 -> head0, 32+96 -> head1)
                    for i in range(2):
                        c1 = smallp.tile([1, MCH], f32, tag="c1", name=f"c1{i}")
                        nc.vector.tensor_copy(c1, d_ps[64 + i * 32 : 64 + i * 32 + 1, :])
                        ds = smallp.tile([1, MCH], f32, tag="ds", name=f"ds{i}")
                        nc.vector.tensor_add(ds, d_ps[i * 32 : i * 32 + 1, :], c1)
                        rd = smallp.tile([1, MCH], f32, tag="rd", name=f"rd{i}")
                        nc.vector.reciprocal_approx_fast(out=rd, in_=ds)
                        if DEBUG:
                            row = (mc * NP + hp) * 2 + i
                            nc.sync.dma_start(out=dbg_d[row : row + 1, :], in_=ds)
                            oc = smallp.tile([1, MCH], f32, tag="oc", name=f"oc{i}")
                            nc.vector.tensor_copy(oc, o_ps[i * 64 : i * 64 + 1, :])
                            nc.sync.dma_start(out=dbg_o[row : row + 1, :], in_=oc)
                        bd = bpool.tile([64, MCH], f32, tag="bd", name=f"bd{i}")
                        nc.gpsimd.partition_broadcast(bd, rd)
                        nc.vector.tensor_mul(
                            u_sb[hp][i * 64 : (i + 1) * 64, mc * MCH : (mc + 1) * MCH],
                            o_ps[i * 64 : (i + 1) * 64, :],
                            bd,
                        )
                for mt in range(mc * 4, mc * 4 + 4):
                    lazy.append(lambda mt=mt: outproj_mt(mt))
            while deferred:
                deferred.pop(0)()
            while lazy:
                lazy.pop(0)()
            if DEBUG:
                nc.sync.dma_start(out=dbg_u[:, :], in_=u_sb[0])
                nc.sync.dma_start(out=dbg_q[:, :], in_=qT2[0])
                nc.sync.dma_start(out=dbg_k[:, :], in_=kT2[0])

    nc.finalize()
    return nc


_NC_CACHE = {}


def _get_nc(key=(1024, 4096, 512, 4, 64)):
    if key not in _NC_CACHE:
        _NC_CACHE[key] = build_core(*key)
    return _NC_CACHE[key]


def kernel(left, right, mask, Wq, Wkv, Wout, bout):
    global LAST_RESULTS
    B, M, DQmat = left.shape
    _, N, DC = right.shape
    H, DH = 8, 64
    D = H * DH
    Hc = H // 2
    scale = DH ** -0.5

    left = np.asarray(left, dtype=np.float32)
    right = np.asarray(right, dtype=np.float32)
    Wq = np.asarray(Wq, dtype=np.float32)
    Wkv = np.asarray(Wkv, dtype=np.float32)
    Wout = np.asarray(Wout, dtype=np.float32)
    bout = np.asarray(bout, dtype=np.float32)

    Wqs = (Wq * scale).astype(BF16)
    Wk = Wkv[:, :D].astype(BF16)
    Wv = Wkv[:, D:].astype(BF16)
    WoutB = Wout.astype(BF16)

    leftT = np.ascontiguousarray(left.transpose(0, 2, 1)).astype(BF16)
    rightT = np.ascontiguousarray(right.transpose(0, 2, 1)).astype(BF16)
    maskT = np.ascontiguousarray(mask.transpose(0, 2, 1)).astype(BF16)

    nc = _get_nc((M, N, DQmat, Hc, DH))

    in_maps = []
    for core in range(8):
        b, hh = divmod(core, 2)
        hs = slice(hh * Hc * DH, (hh + 1) * Hc * DH)
        in_maps.append(
            {
                "leftT": leftT[b],
                "rightT": rightT[b],
                "maskT": maskT[b],
                "wq": np.ascontiguousarray(Wqs[:, hs]),
                "wk": np.ascontiguousarray(Wk[:, hs]),
                "wv": np.ascontiguousarray(Wv[:, hs]),
                "wout": np.ascontiguousarray(WoutB[hs, :]),
            }
        )

    tmpdir = None
    if TRACE:
        import shutil

        shutil.rmtree("/tmp/attn_trace", ignore_errors=True)
        tmpdir = "/tmp/attn_trace"
    res = run_bass_kernel_spmd(nc, in_maps, list(range(8)), trace=TRACE, tmpdir=tmpdir)
    LAST_RESULTS = res

    out = np.zeros((B, M, DQmat), np.float32)
    for core in range(8):
        out[core // 2] += res.results[core]["out_p"]
    out += bout[None, None, :]
    return out
